# revision 54
# baseline (speedup 1.0000x reference)
"""Cross-attention Trainium2 kernel (bf16 PE pipeline, fp8 scores).

Problem: B=8, SQ=SKV=2048, HIDDEN=256, fp32.
  Q = query @ Wq.T + bq ; K = key @ Wk.T + bk ; V = value @ Wv.T + bv
  out = softmax(Q @ K.T / sqrt(128)) @ V

Sharding: data-parallel over batch — one batch element per NeuronCore,
8 cores, no collectives. Activations are passed to the device in [d, s]
bf16 layout (cast + transposed on the host as part of sharding/layout
prep); weights likewise pre-transposed [d, e] bf16.

Optimizations over the 84.5us baseline (NTFF-profile driven; measured
75.9us on unthrottled silicon — note the chip P0-downclocks the PE to
~2.0 GHz under sustained multi-hour load, inflating any measurement by
~15%):
  * The ACT exp stream (32 x [128,1024] exps at ~1.15us each) is the
    longest serial chain; it begins at qt = bias(wq @ qtr0).  wq/qtr0
    are therefore loaded FIRST (split by contraction half across the
    sync/scalar HWDGE queues so the halves stream in parallel), and
    qb0's qproj is hoisted ahead of kproj0.  Under the 8-core head
    crunch DMA completions lag issue by 2.5-6us, so this ordering is
    worth ~6us of exp-stream start time.
  * kproj runs in fp8 DoubleRow (wk host-cast to fp8e4; key input was
    already fp8): one matmul per ec contracts both d-halves.  numpy
    sim: rel-err 1.56e-2 -> 1.67e-2 (HW: 1.30e-2 -> 1.51e-2) vs the
    2e-2 gate.
  * kproj(1..3) are spread through qb0's score loop just ahead of the
    score pairs that read their KT blocks; vprojs sit AFTER qb0's
    scores (value blocks land 10-25us in, and a vproj matmul parked in
    the static PE order head-of-line blocks the score->exp stream on a
    DMA scores don't need).
  * AV(qb-1) is interleaved into qb's score loop as HALF groups (8 of
    16 k-chunks per score pair) so the PE burst between score pairs
    stays ~0.9us and the exp feed tracks its dependency-earliest
    schedule.
  * kproj/vproj/qproj accumulate in a small dedicated PSUM pool (ps_sm)
    so their tile allocation never gates on the exp stream draining
    ps_a; u ring deepened to 20 so exps never wait on AV still reading
    the previous block's u tiles; warm-spin matmuls write into the
    ps_av pool (AV starts late; frees a PSUM bank for ps_sm).
  * The out normalization runs on DVE (reciprocal + tensor_scalar
    mult), with the scalar bounced through a GPSIMD tensor_copy — see
    the note in emit_av_evict — keeping ACT dedicated to exps.  The
    final block's evictions revert to ACT (idle by then, shorter
    chain).
  * A dummy 1-element ACTIVATE early on the scalar queue makes
    walrus's exp ACT_TABLE_LOAD (~1.3us) run in the head's DMA shadow
    instead of delaying the first real exp; Vp ones-columns are
    memset-initialized (drops the bv DMA dependency); warm spins cover
    the HAM clock-gate window (PE runs at 1.2 GHz until ~3.4us of
    sustained activity) and the head DMA-arrival window.

Per-core pipeline (all matmul PSUM accumulation fp32):
  P:  projections.  K^T[e,k] and Q^T[e,q] come out of the PE in
      transposed layout; bias added on DVE during PSUM->SBUF eviction
      (bf16 or fp8 out).  V stays natural [k,e]; bv added by DVE with a
      partition-broadcast bias tile into V' (bf16) which carries two
      extra all-ones columns (col 256 = softmax denominator, col 257
      pads the free dim to an even size).
  S:  S^T[k,q] per 512-wide q block; exp(x/SCALE) fused into the ACT
      PSUM->SBUF eviction, bf16 out.  No max-subtraction: scores are
      ~N(0,0.5) by construction.
  A:  numerator AND denominator in one matmul: U.T @ V' with the ones
      column giving psum col 256 = sum_k exp.  Final: out =
      psum[:, :256] * reciprocal(col 256), reciprocal on DVE, multiply
      on ACT (DVE scalar-consumer-after-reciprocal crashes the device).
"""

import numpy as np

B, SQ, SKV, H = 8, 2048, 2048, 256
SCALE = float(np.sqrt(H / 2.0))
N_CORES = 8

P = 128          # partitions
DC = H // P      # d chunks (2)
EC = H // P      # e chunks (2)
NB = SQ // 512   # 512-row seq blocks (4)
KC = SKV // P    # k chunks (16)

USE_FP8_SCORES = True

_CACHE: dict = {}


def _emit(ctx, tc, aps):
    from concourse import mybir

    nc = tc.nc
    f32 = mybir.dt.float32
    bf16 = mybir.dt.bfloat16
    f8 = mybir.dt.float8e4
    qk_dt = f8 if USE_FP8_SCORES else bf16
    AF = mybir.ActivationFunctionType
    queryT, keyT, valueT, wqT, wkT, wvT, bq2, bk2, bvr, out = aps
    inv_scale = 1.0 / SCALE

    const_pool = ctx.enter_context(tc.tile_pool(name="const", bufs=1))
    kin_pool = ctx.enter_context(tc.tile_pool(name="kin", bufs=4))
    qin_pool = ctx.enter_context(tc.tile_pool(name="qin", bufs=3))
    vin_pool = ctx.enter_context(tc.tile_pool(name="vin", bufs=NB))
    ktv_pool = ctx.enter_context(tc.tile_pool(name="ktv", bufs=1))
    qt_pool = ctx.enter_context(tc.tile_pool(name="qt", bufs=2))
    # u ring must be deep enough that block qb's exps never wait on
    # AV(qb-1) still reading its u tiles: AV(qb-1) finishes reading only
    # at the END of qb's score loop, so depth must cover 2 full blocks
    # (16) plus slack — at 12, exps kp4..7 of every block stalled on the
    # previous block's AV reads, starving ACT (measured 26us of ACT gaps).
    u_pool = ctx.enter_context(tc.tile_pool(name="u", bufs=20))
    out_pool = ctx.enter_context(tc.tile_pool(name="outp", bufs=3))
    rec_pool = ctx.enter_context(tc.tile_pool(name="rec", bufs=3))
    ps_a = ctx.enter_context(tc.tile_pool(name="ps_a", bufs=2, space="PSUM"))
    ps_sm = ctx.enter_context(tc.tile_pool(name="ps_sm", bufs=2, space="PSUM"))
    ps_av = ctx.enter_context(tc.tile_pool(name="ps_av", bufs=2, space="PSUM"))

    # warm tile memset first in the DVE program: it gates the PE warm-up
    # spins, which should start as soon as the DVE queue comes up.
    warm = const_pool.tile([P, 64], bf16, tag="warm")
    nc.vector.memset(warm, 0.0)

    # ---- input DMA schedule ----
    # Explicit first-use ordering across the two HWDGE queues (sync,
    # scalar) and the SWDGE queue (gpsimd).  Each HWDGE queue entry
    # occupies its queue for roughly the transfer time, so a late-needed
    # block queued early delays every block behind it (v1's value-blk0
    # sat behind weights+key on scalar and stalled the PE 3.2us).
    def weight_tile(name, src_ap, dma):
        w = const_pool.tile([P, DC, H], bf16, tag=name)
        dma.dma_start(w, src_ap.rearrange("(c p) e -> p c e", p=P))
        return w

    def load_T(src, blk, dma, tag, pool, dt=bf16):
        """DMA a 512-col block of a [H, seq] dram tensor into a
        [d_part, dc, 512] SBUF tile (contiguous rows per partition)."""
        t = pool.tile([P, DC, 512], dt, tag=tag)
        dma.dma_start(
            t, src[:, blk * 512:(blk + 1) * 512].rearrange("(c p) s -> p c s", p=P)
        )
        return t

    # scalar queue: wk, bk, bq, wq, [dummy act -> table load], value1,
    #               key3, value3
    # sync queue:   key0, query0, key1, value0, key2, value2
    # gpsimd queue: wv, bv
    # The exp stream is the kernel's longest serial chain and it begins
    # at qt = bias(wq @ qtr0): those inputs go FIRST on both HWDGE
    # queues, split by contraction half so they stream in parallel.
    # key/wk follow — kproj only has to beat the first score matmul.
    wq_sb = const_pool.tile([P, DC, H], bf16, tag="wq")
    wq_src = wqT.rearrange("(c p) e -> p c e", p=P)
    qtr0 = qin_pool.tile([P, DC, 512], bf16, tag="qin")
    q0_src = queryT[:, 0:512].rearrange("(c p) s -> p c s", p=P)
    nc.sync.dma_start(qtr0[:, 0:1, :], q0_src[:, 0:1, :])
    nc.scalar.dma_start(qtr0[:, 1:2, :], q0_src[:, 1:2, :])
    ktrs = [None] * NB
    ktrs[0] = load_T(keyT, 0, nc.sync, "kin", kin_pool, dt=f8)
    nc.scalar.dma_start(wq_sb[:, 1:2, :], wq_src[:, 1:2, :])
    nc.sync.dma_start(wq_sb[:, 0:1, :], wq_src[:, 0:1, :])
    bq_sb = const_pool.tile([P, EC], f32)
    nc.scalar.dma_start(bq_sb, bq2.rearrange("c p -> p c"))
    # wk is fp8 (64KB): ride the sync queue right behind key0 so the
    # kproj0 chain completes at the same time as the qt chain — the
    # first score matmul needs BOTH, and wk parked behind the scalar
    # queue's bulk made kproj0 the laggard by ~3us.
    wk_sb = const_pool.tile([P, DC, H], f8, tag="wk")
    nc.sync.dma_start(wk_sb, wkT.rearrange("(c p) e -> p c e", p=P))
    bk_sb = const_pool.tile([P, EC], f32)
    nc.scalar.dma_start(bk_sb, bk2.rearrange("c p -> p c"))
    wv_sb = weight_tile("wv", wvT, nc.gpsimd)
    bv_row = const_pool.tile([1, H], f32)
    nc.gpsimd.dma_start(bv_row, bvr)
    ktrs[1] = load_T(keyT, 1, nc.sync, "kin", kin_pool, dt=f8)
    # dummy activation: forces the exp ACT_TABLE_LOAD to run here, in the
    # head's DMA shadow, instead of in front of the first real exp.
    dumm = const_pool.tile([1, 2], f32, tag="dumm")
    nc.vector.memset(dumm, 0.0)
    nc.scalar.activation(dumm, dumm, AF.Exp, scale=1.0)
    vtrs = [None] * NB
    vtrs[0] = load_T(valueT, 0, nc.sync, "vin", vin_pool)
    vtrs[1] = load_T(valueT, 1, nc.scalar, "vin", vin_pool)
    ktrs[2] = load_T(keyT, 2, nc.sync, "kin", kin_pool, dt=f8)
    ktrs[3] = load_T(keyT, 3, nc.scalar, "kin", kin_pool, dt=f8)
    vtrs[2] = load_T(valueT, 2, nc.sync, "vin", vin_pool)
    vtrs[3] = load_T(valueT, 3, nc.scalar, "vin", vin_pool)

    bv_rep = const_pool.tile([P, H], f32)
    nc.gpsimd.partition_broadcast(bv_rep, bv_row)

    # ---- persistent per-core tensors ----
    KT = ktv_pool.tile([P, EC, SKV], qk_dt)    # [e_part, ec, k]
    # V' carries 2 extra columns of ones: col 256 is the softmax
    # denominator; col 257 pads the matmul free dim to an even size.
    Vp = ktv_pool.tile([P, KC, H + 2], bf16)   # [k_part, kc, e | ones ones]
    nc.vector.memset(Vp[:, :, H:H + 2], 1.0)

    # ---- PE warm-up spin ----
    # The HAM clock gate starts at K=4/8 (1.2 GHz) and only releases to
    # 2.4 GHz after ~3.4us of sustained PE activity.  The PE would
    # otherwise sit idle waiting for the first K/weight DMAs, then run
    # the first ~3.4us of real matmuls at half clock.  Spin cheap dummy
    # matmuls (no DMA dependencies) through the warm-up window instead.
    # spins write into the ps_av pool: AV work starts ~15us in, so the
    # head-phase spins never contend with a live AV accumulation.
    pw = ps_av.tile([P, H + 2], f32, tag="ps_av")

    def warm_spin(n):
        # Cheap dependency-free matmuls emitted just before DMA-gated real
        # work: they soak up what would be PE idle (keeping the HAM window
        # busy) and cost ~50ns each when the real work is actually ready.
        for _ in range(n):
            nc.tensor.matmul(pw[0:64, 0:64], lhsT=warm, rhs=warm, start=True, stop=True)

    # qtr0/wq/k0/wk land ~6-7us in under the 8-core head crunch (stable
    # across every measured run); blanket that window with spins so the
    # HAM activity monitor sees a continuously busy PE and un-throttles
    # at ~3.4us.  These spins precede the DMA-gated qproj in the queue,
    # so they fill what would be idle; if arrivals ever came early the
    # worst case is a fraction of one spin block (~0.4us).
    warm_spin(104)

    def emit_qproj(qtr):
        # qproj accumulates in ps_sm (not ps_a): the ps_a ring buffers are
        # freed by ACT exp evictions, which lag the PE by ~1 exp at every
        # q-block boundary and were stalling the PE ~0.8us per block.
        qt = qt_pool.tile([P, EC, 512], qk_dt, tag="qt")   # [e_part, ec, q]
        for ec in range(EC):
            pq = ps_sm.tile([P, 512], f32, tag="ps_sm")
            for dc in range(DC):
                nc.tensor.matmul(
                    pq,
                    lhsT=wq_sb[:, dc, ec * P:(ec + 1) * P],
                    rhs=qtr[:, dc, :],
                    start=(dc == 0),
                    stop=(dc == DC - 1),
                )
            nc.vector.tensor_scalar(
                qt[:, ec, :],
                pq,
                bq_sb[:, ec:ec + 1], None, mybir.AluOpType.add,
            )
        return qt

    # qb0's projection is hoisted ahead of kproj0: its inputs arrive
    # first and it heads the exp-stream critical chain.
    qt0 = emit_qproj(qtr0)

    def emit_kproj(blk):
        # per-ec [P,512] accumulation in the small pool: keeps kproj's
        # PSUM allocation off the ps_a ring (whose buffers are only freed
        # by ACT exp evictions, which would in-order-stall the PE here).
        # fp8 DoubleRow: key input is already fp8 and wk is host-cast to
        # fp8 — one matmul contracts both dc planes (numpy sim: rel-err
        # 1.56e-2 -> 1.67e-2, HW headroom vs the 2e-2 gate is ample).
        # K's fp8 rounding feeds the score matmuls that are fp8 anyway.
        for ec in range(EC):
            pk = ps_sm.tile([P, 512], f32, tag="ps_sm")
            nc.tensor.matmul(
                pk,
                lhsT=wk_sb[:, :, ec * P:(ec + 1) * P],
                rhs=ktrs[blk][:, :, :],
                start=True,
                stop=True,
                perf_mode=mybir.MatmulPerfMode.DoubleRow,
            )
            nc.vector.tensor_scalar(
                KT[:, ec, blk * 512:(blk + 1) * 512],
                pk,
                bk_sb[:, ec:ec + 1], None, mybir.AluOpType.add,
            )

    emit_kproj(0)
    warm_spin(8)

    def emit_vproj(kc):
        blk, j = kc // 4, kc % 4
        pv = ps_sm.tile([P, 512], f32, tag="ps_sm")
        for dc in range(DC):
            nc.tensor.matmul(
                pv[:, 0:H],
                lhsT=vtrs[blk][:, dc, j * P:(j + 1) * P],
                rhs=wv_sb[:, dc, :],
                start=(dc == 0),
                stop=(dc == DC - 1),
            )
        nc.vector.tensor_add(Vp[:, kc, 0:H], pv[:, 0:H], bv_rep)

    # ---- query blocks: project, scores+exp, AV, finalize ----
    # Software-pipelined emission: the PE instruction queue is strictly
    # in-order, so the AV matmuls of block qb-1 (whose U tiles exist) are
    # interleaved between the score-tile fills of block qb.  That way the
    # PE never sits in-order-blocked behind an ACT exp it doesn't depend
    # on, and ACT's exp stream drains while the PE chews AV work.
    def emit_av_half(us, qs, half, pav=None):
        """Accumulate 8 of the 16 kc chunks into pav.  Split so the PE
        burst between score pairs stays ~0.9us — the static scheduler
        otherwise parks full 1.8us AV groups in front of the score
        matmuls that feed the (starving) ACT exp stream."""
        if pav is None:
            pav = ps_av.tile([P, H + 2], f32, tag="ps_av")
        for kc in range(half * 8, half * 8 + 8):
            u2 = us[kc // 2]
            off = (kc % 2) * 512
            nc.tensor.matmul(
                pav,
                lhsT=u2[:, off + qs * P: off + (qs + 1) * P],
                rhs=Vp[:, kc, :],
                start=(kc == 0),
                stop=(kc == KC - 1),
            )
        return pav

    def emit_av_evict(pav, qb, qs, last=False):
        ot = out_pool.tile([P, H], f32, tag="ot")
        # NB: vector.tensor_scalar_mul reading a scalar that DVE's
        # reciprocal just produced crashes the device (observed
        # NRT_EXEC_UNIT_UNRECOVERABLE) — the scalar operand is fetched at
        # dispatch time, and same-engine program order doesn't guard that
        # fetch against the in-flight producer.  Bounce the scalar through
        # the (idle) GPSIMD engine instead: the cross-engine hop gets a
        # real semaphore that gates the consumer's dispatch, and the
        # multiply itself runs on DVE, keeping ACT free for the exp
        # stream (ACT was within ~3% of the PE's critical path).
        rec = rec_pool.tile([P, 1], f32, tag="rec")
        nc.vector.reciprocal(rec, pav[:, H:H + 1])
        if last:
            # ACT has drained its exp backlog by the final block, and its
            # rec->scale chain is ~2 hops shorter than the GPSIMD bounce —
            # use it for the latency-critical tail evictions.
            nc.scalar.activation(ot, pav[:, 0:H], AF.Copy, scale=rec)
        else:
            rec2 = rec_pool.tile([P, 1], f32, tag="rec2")
            nc.gpsimd.tensor_copy(rec2, rec)
            nc.vector.tensor_scalar(
                ot, pav[:, 0:H], rec2, None, mybir.AluOpType.mult
            )
        # Final slice: issue its out-DMA from the scalar queue, right
        # behind its own ACT scale — same-queue ordering drops the
        # cross-engine semaphore hops from the very last store's chain.
        # Earlier final-block slices go via sync so their transfers run
        # in parallel with the remaining scalar-queue evictions.
        dma = nc.scalar if (last and qs == 3) else nc.sync
        dma.dma_start(
            out[qb * 512 + qs * P: qb * 512 + (qs + 1) * P, :], ot
        )

    prev_us = None
    for qb in range(NB):
        if qb == 0:
            qt = qt0
        else:
            qtr = load_T(queryT, qb, nc.sync, "qin", qin_pool)
            qt = emit_qproj(qtr)

        # scores S^T[k, q] for this q block, exp'ed into U tiles (bf16),
        # with the previous block's AV work interleaved
        us = []
        for kp in range(KC // 2):
            pst = ps_a.tile([P, 1024], f32, tag="ps_a")
            for hh in range(2):
                kc = kp * 2 + hh
                if USE_FP8_SCORES:
                    nc.tensor.matmul(
                        pst[:, hh * 512:(hh + 1) * 512],
                        lhsT=KT[:, :, kc * P:(kc + 1) * P],
                        rhs=qt[:, :, :],
                        start=True,
                        stop=True,
                        perf_mode=mybir.MatmulPerfMode.DoubleRow,
                    )
                else:
                    for ec in range(EC):
                        nc.tensor.matmul(
                            pst[:, hh * 512:(hh + 1) * 512],
                            lhsT=KT[:, ec, kc * P:(kc + 1) * P],
                            rhs=qt[:, ec, :],
                            start=(ec == 0),
                            stop=(ec == EC - 1),
                        )
            u2 = u_pool.tile([P, 1024], bf16, tag="u2")
            nc.scalar.activation(u2, pst, AF.Exp, scale=inv_scale)
            us.append(u2)
            if qb == 0:
                # Interleave the remaining K projections so each is
                # emitted just before the first score matmul that needs
                # its KT block.  The V projections are NOT interleaved
                # here: value blocks land 10-25us in under the head
                # crunch, and a vproj matmul parked in the static PE
                # order ahead of a score matmul head-of-line blocks the
                # score -> exp stream on a DMA that scores don't need.
                # All vprojs are emitted after this loop instead; the
                # scheduler hoists them into the exp-gated PE bubbles,
                # and AV (their only consumer) starts a block later.
                if kp == 1:
                    emit_kproj(1)
                elif kp == 3:
                    emit_kproj(2)
                elif kp == 5:
                    emit_kproj(3)
            else:
                # one half-AV (8 chunks, ~0.9us) per score pair: kp1
                # starts qs0, kp2 finishes it, ..., kp7 starts qs3 whose
                # second half lands right after the loop.
                if kp % 2 == 1:
                    pend_pav = emit_av_half(prev_us, kp // 2, 0)
                elif kp >= 2:
                    qs = (kp - 2) // 2
                    emit_av_half(prev_us, qs, 1, pav=pend_pav)
                    emit_av_evict(pend_pav, qb - 1, qs)
        if qb == 0:
            # vprojs sit between qb0's scores and their first consumer
            # (AV(qb0) half 0 at qb1-kp1); by now every value block has
            # landed, and the PE chews them while ACT drains qb0's exps.
            for kc in range(KC):
                emit_vproj(kc)
        else:
            emit_av_half(prev_us, 3, 1, pav=pend_pav)
            emit_av_evict(pend_pav, qb - 1, 3)
        prev_us = us

    for qs in range(4):
        pav = emit_av_half(prev_us, qs, 0)
        emit_av_half(prev_us, qs, 1, pav=pav)
        # qs0/qs1 evict via the DVE+GPSIMD path, qs2/qs3 via ACT: the two
        # chains drain in parallel so the tail isn't 4 serial ACT scales.
        emit_av_evict(pav, NB - 1, qs, last=(qs >= 2))


def _build():
    from contextlib import ExitStack

    import concourse.tile as tile
    from concourse import bacc, mybir

    f32 = mybir.dt.float32
    bf16 = mybir.dt.bfloat16
    nc = bacc.Bacc(
        "TRN2", target_bir_lowering=False, debug=False, num_devices=N_CORES
    )
    queryT = nc.dram_tensor("queryT", [H, SQ], bf16, kind="ExternalInput").ap()
    keyT = nc.dram_tensor(
        "keyT", [H, SKV], mybir.dt.float8e4, kind="ExternalInput"
    ).ap()
    valueT = nc.dram_tensor("valueT", [H, SKV], bf16, kind="ExternalInput").ap()
    wqT = nc.dram_tensor("wqT", [H, H], bf16, kind="ExternalInput").ap()
    wkT = nc.dram_tensor(
        "wkT", [H, H], mybir.dt.float8e4, kind="ExternalInput"
    ).ap()
    wvT = nc.dram_tensor("wvT", [H, H], bf16, kind="ExternalInput").ap()
    bq2 = nc.dram_tensor("bq2", [EC, P], f32, kind="ExternalInput").ap()
    bk2 = nc.dram_tensor("bk2", [EC, P], f32, kind="ExternalInput").ap()
    bvr = nc.dram_tensor("bvr", [1, H], f32, kind="ExternalInput").ap()
    out = nc.dram_tensor("out", [SQ, H], f32, kind="ExternalOutput").ap()

    aps = (queryT, keyT, valueT, wqT, wkT, wvT, bq2, bk2, bvr, out)
    with tile.TileContext(nc) as tc, ExitStack() as ctx:
        _emit(ctx, tc, aps)
    nc.compile()
    return nc


def _get_nc():
    if "nc" not in _CACHE:
        _CACHE["nc"] = _build()
    return _CACHE["nc"]


def _in_maps(query, key, value, Wq, bq, Wk, bk, Wv, bv):
    import ml_dtypes

    bf16 = ml_dtypes.bfloat16
    q = np.asarray(query, np.float32)
    k = np.asarray(key, np.float32)
    v = np.asarray(value, np.float32)
    # [B, s, d] -> [B, d, s] bf16 layout prep for the device (contraction
    # dim on partitions), done host-side as part of sharding.
    qT = np.ascontiguousarray(q.transpose(0, 2, 1)).astype(bf16)
    kT = np.ascontiguousarray(k.transpose(0, 2, 1)).astype(ml_dtypes.float8_e4m3fn)
    vT = np.ascontiguousarray(v.transpose(0, 2, 1)).astype(bf16)
    wqT = np.ascontiguousarray(np.asarray(Wq, np.float32).T).astype(bf16)
    wkT = np.ascontiguousarray(np.asarray(Wk, np.float32).T).astype(
        ml_dtypes.float8_e4m3fn
    )
    wvT = np.ascontiguousarray(np.asarray(Wv, np.float32).T).astype(bf16)
    bq2 = np.ascontiguousarray(np.asarray(bq, np.float32).reshape(EC, P))
    bk2 = np.ascontiguousarray(np.asarray(bk, np.float32).reshape(EC, P))
    bvr = np.ascontiguousarray(np.asarray(bv, np.float32).reshape(1, H))
    maps = []
    for b in range(B):
        maps.append(
            {
                "queryT": qT[b],
                "keyT": kT[b],
                "valueT": vT[b],
                "wqT": wqT,
                "wkT": wkT,
                "wvT": wvT,
                "bq2": bq2,
                "bk2": bk2,
                "bvr": bvr,
            }
        )
    return maps


def _run(in_maps, trace=False, **kw):
    import concourse.bass_utils as bass_utils

    if trace:
        # zero-egress container: skip the artifact upload step
        bass_utils.upload_artifacts = lambda tmpdir: f"local://{tmpdir}"
    nc = _get_nc()
    return bass_utils.run_bass_kernel_spmd(
        nc, in_maps, list(range(N_CORES)), trace=trace, **kw
    )


def kernel(query, key, value, Wq, bq, Wk, bk, Wv, bv):
    res = _run(_in_maps(query, key, value, Wq, bq, Wk, bk, Wv, bv))
    return np.stack([res.results[b]["out"] for b in range(B)], axis=0)


# revision 56
# speedup vs baseline: 1.0044x; 1.0044x over previous
"""Cross-attention Trainium2 kernel (bf16 PE pipeline, fp8 scores).

Problem: B=8, SQ=SKV=2048, HIDDEN=256, fp32.
  Q = query @ Wq.T + bq ; K = key @ Wk.T + bk ; V = value @ Wv.T + bv
  out = softmax(Q @ K.T / sqrt(128)) @ V

Sharding: data-parallel over batch — one batch element per NeuronCore,
8 cores, no collectives. Activations are passed to the device in [d, s]
bf16 layout (cast + transposed on the host as part of sharding/layout
prep); weights likewise pre-transposed [d, e] bf16.

Optimizations over the 84.5us baseline (NTFF-profile driven; measured
75.9us on unthrottled silicon — note the chip P0-downclocks the PE to
~2.0 GHz under sustained multi-hour load, inflating any measurement by
~15%):
  * The ACT exp stream (32 x [128,1024] exps at ~1.15us each) is the
    longest serial chain; it begins at qt = bias(wq @ qtr0).  wq/qtr0
    are therefore loaded FIRST (split by contraction half across the
    sync/scalar HWDGE queues so the halves stream in parallel), and
    qb0's qproj is hoisted ahead of kproj0.  Under the 8-core head
    crunch DMA completions lag issue by 2.5-6us, so this ordering is
    worth ~6us of exp-stream start time.
  * kproj runs in fp8 DoubleRow (wk host-cast to fp8e4; key input was
    already fp8): one matmul per ec contracts both d-halves.  numpy
    sim: rel-err 1.56e-2 -> 1.67e-2 (HW: 1.30e-2 -> 1.51e-2) vs the
    2e-2 gate.
  * kproj(1..3) are spread through qb0's score loop just ahead of the
    score pairs that read their KT blocks; vprojs sit AFTER qb0's
    scores (value blocks land 10-25us in, and a vproj matmul parked in
    the static PE order head-of-line blocks the score->exp stream on a
    DMA scores don't need).
  * AV(qb-1) is interleaved into qb's score loop as HALF groups (8 of
    16 k-chunks per score pair) so the PE burst between score pairs
    stays ~0.9us and the exp feed tracks its dependency-earliest
    schedule.
  * kproj/vproj/qproj accumulate in a small dedicated PSUM pool (ps_sm)
    so their tile allocation never gates on the exp stream draining
    ps_a; u ring deepened to 20 so exps never wait on AV still reading
    the previous block's u tiles; warm-spin matmuls write into the
    ps_av pool (AV starts late; frees a PSUM bank for ps_sm).
  * The out normalization runs on DVE (reciprocal + tensor_scalar
    mult), with the scalar bounced through a GPSIMD tensor_copy — see
    the note in emit_av_evict — keeping ACT dedicated to exps.  The
    final block's evictions revert to ACT (idle by then, shorter
    chain).
  * A dummy 1-element ACTIVATE early on the scalar queue makes
    walrus's exp ACT_TABLE_LOAD (~1.3us) run in the head's DMA shadow
    instead of delaying the first real exp; Vp ones-columns are
    memset-initialized (drops the bv DMA dependency); warm spins cover
    the HAM clock-gate window (PE runs at 1.2 GHz until ~3.4us of
    sustained activity) and the head DMA-arrival window.

Per-core pipeline (all matmul PSUM accumulation fp32):
  P:  projections.  K^T[e,k] and Q^T[e,q] come out of the PE in
      transposed layout; bias added on DVE during PSUM->SBUF eviction
      (bf16 or fp8 out).  V stays natural [k,e]; bv added by DVE with a
      partition-broadcast bias tile into V' (bf16) which carries two
      extra all-ones columns (col 256 = softmax denominator, col 257
      pads the free dim to an even size).
  S:  S^T[k,q] per 512-wide q block; exp(x/SCALE) fused into the ACT
      PSUM->SBUF eviction, bf16 out.  No max-subtraction: scores are
      ~N(0,0.5) by construction.
  A:  numerator AND denominator in one matmul: U.T @ V' with the ones
      column giving psum col 256 = sum_k exp.  Final: out =
      psum[:, :256] * reciprocal(col 256), reciprocal on DVE, multiply
      on ACT (DVE scalar-consumer-after-reciprocal crashes the device).
"""

import numpy as np

B, SQ, SKV, H = 8, 2048, 2048, 256
SCALE = float(np.sqrt(H / 2.0))
N_CORES = 8

P = 128          # partitions
DC = H // P      # d chunks (2)
EC = H // P      # e chunks (2)
NB = SQ // 512   # 512-row seq blocks (4)
KC = SKV // P    # k chunks (16)

USE_FP8_SCORES = True

_CACHE: dict = {}


def _emit(ctx, tc, aps):
    from concourse import mybir

    nc = tc.nc
    f32 = mybir.dt.float32
    bf16 = mybir.dt.bfloat16
    f8 = mybir.dt.float8e4
    qk_dt = f8 if USE_FP8_SCORES else bf16
    AF = mybir.ActivationFunctionType
    queryT, keyT, valueT, wqT, wkT, wvT, bq2, bk2, bvr, out = aps
    inv_scale = 1.0 / SCALE

    const_pool = ctx.enter_context(tc.tile_pool(name="const", bufs=1))
    kin_pool = ctx.enter_context(tc.tile_pool(name="kin", bufs=4))
    qin_pool = ctx.enter_context(tc.tile_pool(name="qin", bufs=3))
    vin_pool = ctx.enter_context(tc.tile_pool(name="vin", bufs=NB))
    ktv_pool = ctx.enter_context(tc.tile_pool(name="ktv", bufs=1))
    qt_pool = ctx.enter_context(tc.tile_pool(name="qt", bufs=2))
    # u ring must be deep enough that block qb's exps never wait on
    # AV(qb-1) still reading its u tiles: AV(qb-1) finishes reading only
    # at the END of qb's score loop, so depth must cover 2 full blocks
    # (16) plus slack — at 12, exps kp4..7 of every block stalled on the
    # previous block's AV reads, starving ACT (measured 26us of ACT gaps).
    u_pool = ctx.enter_context(tc.tile_pool(name="u", bufs=20))
    out_pool = ctx.enter_context(tc.tile_pool(name="outp", bufs=3))
    rec_pool = ctx.enter_context(tc.tile_pool(name="rec", bufs=3))
    ps_a = ctx.enter_context(tc.tile_pool(name="ps_a", bufs=2, space="PSUM"))
    ps_sm = ctx.enter_context(tc.tile_pool(name="ps_sm", bufs=2, space="PSUM"))
    ps_av = ctx.enter_context(tc.tile_pool(name="ps_av", bufs=2, space="PSUM"))

    # warm tile memset first in the DVE program: it gates the PE warm-up
    # spins, which should start as soon as the DVE queue comes up.
    warm = const_pool.tile([P, 64], bf16, tag="warm")
    nc.vector.memset(warm, 0.0)

    # ---- input DMA schedule ----
    # Explicit first-use ordering across the two HWDGE queues (sync,
    # scalar) and the SWDGE queue (gpsimd).  Each HWDGE queue entry
    # occupies its queue for roughly the transfer time, so a late-needed
    # block queued early delays every block behind it (v1's value-blk0
    # sat behind weights+key on scalar and stalled the PE 3.2us).
    def weight_tile(name, src_ap, dma):
        w = const_pool.tile([P, DC, H], bf16, tag=name)
        dma.dma_start(w, src_ap.rearrange("(c p) e -> p c e", p=P))
        return w

    def load_T(src, blk, dma, tag, pool, dt=bf16):
        """DMA a 512-col block of a [H, seq] dram tensor into a
        [d_part, dc, 512] SBUF tile (contiguous rows per partition)."""
        t = pool.tile([P, DC, 512], dt, tag=tag)
        dma.dma_start(
            t, src[:, blk * 512:(blk + 1) * 512].rearrange("(c p) s -> p c s", p=P)
        )
        return t

    # scalar queue: wk, bk, bq, wq, [dummy act -> table load], value1,
    #               key3, value3
    # sync queue:   key0, query0, key1, value0, key2, value2
    # gpsimd queue: wv, bv
    # The exp stream is the kernel's longest serial chain and it begins
    # at qt = bias(wq @ qtr0): those inputs go FIRST on both HWDGE
    # queues, split by contraction half so they stream in parallel.
    # key/wk follow — kproj only has to beat the first score matmul.
    wq_sb = const_pool.tile([P, DC, H], bf16, tag="wq")
    wq_src = wqT.rearrange("(c p) e -> p c e", p=P)
    qtr0 = qin_pool.tile([P, DC, 512], bf16, tag="qin")
    q0_src = queryT[:, 0:512].rearrange("(c p) s -> p c s", p=P)
    nc.sync.dma_start(qtr0[:, 0:1, :], q0_src[:, 0:1, :])
    nc.scalar.dma_start(qtr0[:, 1:2, :], q0_src[:, 1:2, :])
    ktrs = [None] * NB
    ktrs[0] = load_T(keyT, 0, nc.sync, "kin", kin_pool, dt=f8)
    nc.scalar.dma_start(wq_sb[:, 1:2, :], wq_src[:, 1:2, :])
    nc.sync.dma_start(wq_sb[:, 0:1, :], wq_src[:, 0:1, :])
    bq_sb = const_pool.tile([P, EC], f32)
    nc.scalar.dma_start(bq_sb, bq2.rearrange("c p -> p c"))
    # wk is fp8 (64KB): ride the sync queue right behind key0 so the
    # kproj0 chain completes at the same time as the qt chain — the
    # first score matmul needs BOTH, and wk parked behind the scalar
    # queue's bulk made kproj0 the laggard by ~3us.
    wk_sb = const_pool.tile([P, DC, H], f8, tag="wk")
    nc.sync.dma_start(wk_sb, wkT.rearrange("(c p) e -> p c e", p=P))
    bk_sb = const_pool.tile([P, EC], f32)
    nc.scalar.dma_start(bk_sb, bk2.rearrange("c p -> p c"))
    wv_sb = weight_tile("wv", wvT, nc.gpsimd)
    bv_row = const_pool.tile([1, H], f32)
    nc.gpsimd.dma_start(bv_row, bvr)
    ktrs[1] = load_T(keyT, 1, nc.sync, "kin", kin_pool, dt=f8)
    # dummy activation: forces the exp ACT_TABLE_LOAD to run here, in the
    # head's DMA shadow, instead of in front of the first real exp.
    dumm = const_pool.tile([1, 2], f32, tag="dumm")
    nc.vector.memset(dumm, 0.0)
    nc.scalar.activation(dumm, dumm, AF.Exp, scale=1.0)
    vtrs = [None] * NB
    vtrs[0] = load_T(valueT, 0, nc.sync, "vin", vin_pool)
    vtrs[1] = load_T(valueT, 1, nc.scalar, "vin", vin_pool)
    ktrs[2] = load_T(keyT, 2, nc.sync, "kin", kin_pool, dt=f8)
    ktrs[3] = load_T(keyT, 3, nc.scalar, "kin", kin_pool, dt=f8)
    vtrs[2] = load_T(valueT, 2, nc.sync, "vin", vin_pool)
    vtrs[3] = load_T(valueT, 3, nc.scalar, "vin", vin_pool)

    bv_rep = const_pool.tile([P, H], f32)
    nc.gpsimd.partition_broadcast(bv_rep, bv_row)

    # ---- persistent per-core tensors ----
    KT = ktv_pool.tile([P, EC, SKV], qk_dt)    # [e_part, ec, k]
    # V' carries 2 extra columns of ones: col 256 is the softmax
    # denominator; col 257 pads the matmul free dim to an even size.
    Vp = ktv_pool.tile([P, KC, H + 2], bf16)   # [k_part, kc, e | ones ones]
    nc.vector.memset(Vp[:, :, H:H + 2], 1.0)

    # ---- PE warm-up spin ----
    # The HAM clock gate starts at K=4/8 (1.2 GHz) and only releases to
    # 2.4 GHz after ~3.4us of sustained PE activity.  The PE would
    # otherwise sit idle waiting for the first K/weight DMAs, then run
    # the first ~3.4us of real matmuls at half clock.  Spin cheap dummy
    # matmuls (no DMA dependencies) through the warm-up window instead.
    # spins write into the ps_av pool: AV work starts ~15us in, so the
    # head-phase spins never contend with a live AV accumulation.
    pw = ps_av.tile([P, H + 2], f32, tag="ps_av")

    def warm_spin(n):
        # Cheap dependency-free matmuls emitted just before DMA-gated real
        # work: they soak up what would be PE idle (keeping the HAM window
        # busy) and cost ~50ns each when the real work is actually ready.
        for _ in range(n):
            nc.tensor.matmul(pw[0:64, 0:64], lhsT=warm, rhs=warm, start=True, stop=True)

    # qtr0/wq/k0/wk land ~4-6us in under the 8-core head crunch; blanket
    # that window with spins so the HAM activity monitor sees a
    # continuously busy PE and un-throttles at ~3.4us (once warm, sub-
    # 3.4us holes don't re-throttle).
    warm_spin(68)

    def emit_qproj(qtr):
        # qproj accumulates in ps_sm (not ps_a): the ps_a ring buffers are
        # freed by ACT exp evictions, which lag the PE by ~1 exp at every
        # q-block boundary and were stalling the PE ~0.8us per block.
        qt = qt_pool.tile([P, EC, 512], qk_dt, tag="qt")   # [e_part, ec, q]
        for ec in range(EC):
            pq = ps_sm.tile([P, 512], f32, tag="ps_sm")
            for dc in range(DC):
                nc.tensor.matmul(
                    pq,
                    lhsT=wq_sb[:, dc, ec * P:(ec + 1) * P],
                    rhs=qtr[:, dc, :],
                    start=(dc == 0),
                    stop=(dc == DC - 1),
                )
            nc.vector.tensor_scalar(
                qt[:, ec, :],
                pq,
                bq_sb[:, ec:ec + 1], None, mybir.AluOpType.add,
            )
        return qt

    # qb0's projection is hoisted ahead of kproj0: its inputs arrive
    # first and it heads the exp-stream critical chain.
    qt0 = emit_qproj(qtr0)

    def emit_kproj(blk):
        # per-ec [P,512] accumulation in the small pool: keeps kproj's
        # PSUM allocation off the ps_a ring (whose buffers are only freed
        # by ACT exp evictions, which would in-order-stall the PE here).
        # fp8 DoubleRow: key input is already fp8 and wk is host-cast to
        # fp8 — one matmul contracts both dc planes (numpy sim: rel-err
        # 1.56e-2 -> 1.67e-2, HW headroom vs the 2e-2 gate is ample).
        # K's fp8 rounding feeds the score matmuls that are fp8 anyway.
        for ec in range(EC):
            pk = ps_sm.tile([P, 512], f32, tag="ps_sm")
            nc.tensor.matmul(
                pk,
                lhsT=wk_sb[:, :, ec * P:(ec + 1) * P],
                rhs=ktrs[blk][:, :, :],
                start=True,
                stop=True,
                perf_mode=mybir.MatmulPerfMode.DoubleRow,
            )
            nc.vector.tensor_scalar(
                KT[:, ec, blk * 512:(blk + 1) * 512],
                pk,
                bk_sb[:, ec:ec + 1], None, mybir.AluOpType.add,
            )

    emit_kproj(0)
    warm_spin(8)

    def emit_vproj(kc):
        blk, j = kc // 4, kc % 4
        pv = ps_sm.tile([P, 512], f32, tag="ps_sm")
        for dc in range(DC):
            nc.tensor.matmul(
                pv[:, 0:H],
                lhsT=vtrs[blk][:, dc, j * P:(j + 1) * P],
                rhs=wv_sb[:, dc, :],
                start=(dc == 0),
                stop=(dc == DC - 1),
            )
        nc.vector.tensor_add(Vp[:, kc, 0:H], pv[:, 0:H], bv_rep)

    # ---- query blocks: project, scores+exp, AV, finalize ----
    # Software-pipelined emission: the PE instruction queue is strictly
    # in-order, so the AV matmuls of block qb-1 (whose U tiles exist) are
    # interleaved between the score-tile fills of block qb.  That way the
    # PE never sits in-order-blocked behind an ACT exp it doesn't depend
    # on, and ACT's exp stream drains while the PE chews AV work.
    def emit_av_half(us, qs, half, pav=None):
        """Accumulate 8 of the 16 kc chunks into pav.  Split so the PE
        burst between score pairs stays ~0.9us — the static scheduler
        otherwise parks full 1.8us AV groups in front of the score
        matmuls that feed the (starving) ACT exp stream."""
        if pav is None:
            pav = ps_av.tile([P, H + 2], f32, tag="ps_av")
        for kc in range(half * 8, half * 8 + 8):
            u2 = us[kc // 2]
            off = (kc % 2) * 512
            nc.tensor.matmul(
                pav,
                lhsT=u2[:, off + qs * P: off + (qs + 1) * P],
                rhs=Vp[:, kc, :],
                start=(kc == 0),
                stop=(kc == KC - 1),
            )
        return pav

    def emit_av_evict(pav, qb, qs, last=False):
        ot = out_pool.tile([P, H], f32, tag="ot")
        # NB: vector.tensor_scalar_mul reading a scalar that DVE's
        # reciprocal just produced crashes the device (observed
        # NRT_EXEC_UNIT_UNRECOVERABLE) — the scalar operand is fetched at
        # dispatch time, and same-engine program order doesn't guard that
        # fetch against the in-flight producer.  Bounce the scalar through
        # the (idle) GPSIMD engine instead: the cross-engine hop gets a
        # real semaphore that gates the consumer's dispatch, and the
        # multiply itself runs on DVE, keeping ACT free for the exp
        # stream (ACT was within ~3% of the PE's critical path).
        rec = rec_pool.tile([P, 1], f32, tag="rec")
        nc.vector.reciprocal(rec, pav[:, H:H + 1])
        if last:
            # ACT has drained its exp backlog by the final block, and its
            # rec->scale chain is ~2 hops shorter than the GPSIMD bounce —
            # use it for the latency-critical tail evictions.
            nc.scalar.activation(ot, pav[:, 0:H], AF.Copy, scale=rec)
        else:
            rec2 = rec_pool.tile([P, 1], f32, tag="rec2")
            nc.gpsimd.tensor_copy(rec2, rec)
            nc.vector.tensor_scalar(
                ot, pav[:, 0:H], rec2, None, mybir.AluOpType.mult
            )
        # Final slice: issue its out-DMA from the scalar queue, right
        # behind its own ACT scale — same-queue ordering drops the
        # cross-engine semaphore hops from the very last store's chain.
        # Earlier final-block slices go via sync so their transfers run
        # in parallel with the remaining scalar-queue evictions.
        dma = nc.scalar if (last and qs == 3) else nc.sync
        dma.dma_start(
            out[qb * 512 + qs * P: qb * 512 + (qs + 1) * P, :], ot
        )

    prev_us = None
    for qb in range(NB):
        if qb == 0:
            qt = qt0
        else:
            qtr = load_T(queryT, qb, nc.sync, "qin", qin_pool)
            qt = emit_qproj(qtr)

        # scores S^T[k, q] for this q block, exp'ed into U tiles (bf16),
        # with the previous block's AV work interleaved
        us = []
        for kp in range(KC // 2):
            pst = ps_a.tile([P, 1024], f32, tag="ps_a")
            for hh in range(2):
                kc = kp * 2 + hh
                if USE_FP8_SCORES:
                    nc.tensor.matmul(
                        pst[:, hh * 512:(hh + 1) * 512],
                        lhsT=KT[:, :, kc * P:(kc + 1) * P],
                        rhs=qt[:, :, :],
                        start=True,
                        stop=True,
                        perf_mode=mybir.MatmulPerfMode.DoubleRow,
                    )
                else:
                    for ec in range(EC):
                        nc.tensor.matmul(
                            pst[:, hh * 512:(hh + 1) * 512],
                            lhsT=KT[:, ec, kc * P:(kc + 1) * P],
                            rhs=qt[:, ec, :],
                            start=(ec == 0),
                            stop=(ec == EC - 1),
                        )
            u2 = u_pool.tile([P, 1024], bf16, tag="u2")
            nc.scalar.activation(u2, pst, AF.Exp, scale=inv_scale)
            us.append(u2)
            if qb == 0:
                # Interleave the remaining K projections so each is
                # emitted just before the first score matmul that needs
                # its KT block.  The V projections are NOT interleaved
                # here: value blocks land 10-25us in under the head
                # crunch, and a vproj matmul parked in the static PE
                # order ahead of a score matmul head-of-line blocks the
                # score -> exp stream on a DMA that scores don't need.
                # All vprojs are emitted after this loop instead; the
                # scheduler hoists them into the exp-gated PE bubbles,
                # and AV (their only consumer) starts a block later.
                if kp == 1:
                    emit_kproj(1)
                elif kp == 3:
                    emit_kproj(2)
                elif kp == 5:
                    emit_kproj(3)
            else:
                # one half-AV (8 chunks, ~0.9us) per score pair: kp0
                # starts qs0, kp1 finishes it, ..., kp7 finishes qs3.
                # Starting at kp0 (not kp1) removes the post-loop AV
                # spill that delayed the next block's qproj/score head
                # at every q-block boundary.
                if kp % 2 == 0:
                    pend_pav = emit_av_half(prev_us, kp // 2, 0)
                else:
                    emit_av_half(prev_us, kp // 2, 1, pav=pend_pav)
                    emit_av_evict(pend_pav, qb - 1, kp // 2)
        if qb == 0:
            # vprojs sit between qb0's scores and their first consumer
            # (AV(qb0) half 0 at qb1-kp0); by now every value block has
            # landed, and the PE chews them while ACT drains qb0's exps.
            for kc in range(KC):
                emit_vproj(kc)
        prev_us = us

    for qs in range(4):
        pav = emit_av_half(prev_us, qs, 0)
        emit_av_half(prev_us, qs, 1, pav=pav)
        # qs0/qs1 evict via the DVE+GPSIMD path, qs2/qs3 via ACT: the two
        # chains drain in parallel so the tail isn't 4 serial ACT scales.
        emit_av_evict(pav, NB - 1, qs, last=(qs >= 2))


def _build():
    from contextlib import ExitStack

    import concourse.tile as tile
    from concourse import bacc, mybir

    f32 = mybir.dt.float32
    bf16 = mybir.dt.bfloat16
    nc = bacc.Bacc(
        "TRN2", target_bir_lowering=False, debug=False, num_devices=N_CORES
    )
    queryT = nc.dram_tensor("queryT", [H, SQ], bf16, kind="ExternalInput").ap()
    keyT = nc.dram_tensor(
        "keyT", [H, SKV], mybir.dt.float8e4, kind="ExternalInput"
    ).ap()
    valueT = nc.dram_tensor("valueT", [H, SKV], bf16, kind="ExternalInput").ap()
    wqT = nc.dram_tensor("wqT", [H, H], bf16, kind="ExternalInput").ap()
    wkT = nc.dram_tensor(
        "wkT", [H, H], mybir.dt.float8e4, kind="ExternalInput"
    ).ap()
    wvT = nc.dram_tensor("wvT", [H, H], bf16, kind="ExternalInput").ap()
    bq2 = nc.dram_tensor("bq2", [EC, P], f32, kind="ExternalInput").ap()
    bk2 = nc.dram_tensor("bk2", [EC, P], f32, kind="ExternalInput").ap()
    bvr = nc.dram_tensor("bvr", [1, H], f32, kind="ExternalInput").ap()
    out = nc.dram_tensor("out", [SQ, H], f32, kind="ExternalOutput").ap()

    aps = (queryT, keyT, valueT, wqT, wkT, wvT, bq2, bk2, bvr, out)
    with tile.TileContext(nc) as tc, ExitStack() as ctx:
        _emit(ctx, tc, aps)
    nc.compile()
    return nc


def _get_nc():
    if "nc" not in _CACHE:
        _CACHE["nc"] = _build()
    return _CACHE["nc"]


def _in_maps(query, key, value, Wq, bq, Wk, bk, Wv, bv):
    import ml_dtypes

    bf16 = ml_dtypes.bfloat16
    q = np.asarray(query, np.float32)
    k = np.asarray(key, np.float32)
    v = np.asarray(value, np.float32)
    # [B, s, d] -> [B, d, s] bf16 layout prep for the device (contraction
    # dim on partitions), done host-side as part of sharding.
    qT = np.ascontiguousarray(q.transpose(0, 2, 1)).astype(bf16)
    kT = np.ascontiguousarray(k.transpose(0, 2, 1)).astype(ml_dtypes.float8_e4m3fn)
    vT = np.ascontiguousarray(v.transpose(0, 2, 1)).astype(bf16)
    wqT = np.ascontiguousarray(np.asarray(Wq, np.float32).T).astype(bf16)
    wkT = np.ascontiguousarray(np.asarray(Wk, np.float32).T).astype(
        ml_dtypes.float8_e4m3fn
    )
    wvT = np.ascontiguousarray(np.asarray(Wv, np.float32).T).astype(bf16)
    bq2 = np.ascontiguousarray(np.asarray(bq, np.float32).reshape(EC, P))
    bk2 = np.ascontiguousarray(np.asarray(bk, np.float32).reshape(EC, P))
    bvr = np.ascontiguousarray(np.asarray(bv, np.float32).reshape(1, H))
    maps = []
    for b in range(B):
        maps.append(
            {
                "queryT": qT[b],
                "keyT": kT[b],
                "valueT": vT[b],
                "wqT": wqT,
                "wkT": wkT,
                "wvT": wvT,
                "bq2": bq2,
                "bk2": bk2,
                "bvr": bvr,
            }
        )
    return maps


def _run(in_maps, trace=False, **kw):
    import concourse.bass_utils as bass_utils

    if trace:
        # zero-egress container: skip the artifact upload step
        bass_utils.upload_artifacts = lambda tmpdir: f"local://{tmpdir}"
    nc = _get_nc()
    return bass_utils.run_bass_kernel_spmd(
        nc, in_maps, list(range(N_CORES)), trace=trace, **kw
    )


def kernel(query, key, value, Wq, bq, Wk, bk, Wv, bv):
    res = _run(_in_maps(query, key, value, Wq, bq, Wk, bk, Wv, bv))
    return np.stack([res.results[b]["out"] for b in range(B)], axis=0)
